# revision 12
# baseline (speedup 1.0000x reference)
"""CrossEntropyLoss kernel for TRN2 (8 NeuronCores, SPMD).

The reference computes softmax over the full [8192, 32000] matrix but returns
only probs[0, label[0]] = exp(x[0, l0]) / sum_j exp(x[0, j]) -- a scalar that
depends only on row 0 of x and label[0].  The kernel therefore streams just
row 0 (32000 f32, 128KB) to the cores instead of the full 1GB, computes
exp(x_j - x_t) with the target shift folded into the activation bias, row-sums
via the activation accumulator, cross-partition-sums via a ones-matmul on the
PE, and takes the reciprocal on the DVE -- the result is the answer, emitted
by core 0.  Exact for any input; bit-level differences from the reference are
float32 rounding only.

Self-contained: hardcodes shapes, imports only installed packages.
"""

import numpy as np

import concourse.bass as bass
import concourse.bacc as bacc
import concourse.mybir as mybir
from concourse.bass_utils import run_bass_kernel_spmd

M, N = 8192, 32000
P, F = 128, 250  # row 0 reshaped to [128 partitions, 250 free]
N_CORES = 8

_CACHE = {}


def _build():
    # The Bass constructor's all-engine start barrier only orders the
    # const-pool memsets against user code, and the Block-exit barrier only
    # aligns engine halt times.  This program depends on neither: every read
    # is ordered by explicit semaphores (the ones vector ships inside the
    # input DMA), and SP's final out_sem wait guarantees the output landed
    # before the last engine halts.  Dropping both saves ~850ns.
    orig_barrier = bass.Bass.all_engine_barrier
    bass.Bass.all_engine_barrier = lambda self, *, sem_only=False: None
    try:
        nc = bacc.Bacc(
            "TRN2",
            target_bir_lowering=False,
            debug=False,
            num_devices=N_CORES,
        )
        f32 = mybir.dt.float32
        # Input layout: col 0 = -x[0, label[0]] (replicated down the
        # partitions, used as the per-partition activation bias);
        # cols 1..250 = row 0 of x; col 251 = 1.0 (PE reduce vector).
        xin = nc.dram_tensor("xin", [P, F + 2], f32, kind="ExternalInput")
        out = nc.dram_tensor("out", [1, 1], f32, kind="ExternalOutput")

        with (
            nc.Block() as block,
            nc.semaphore("dma_sem") as dma_sem,
            nc.semaphore("act_sem") as act_sem,
            nc.semaphore("pe_sem") as pe_sem,
            nc.semaphore("dve_sem") as dve_sem,
            nc.semaphore("out_sem") as out_sem,
            nc.sbuf_tensor("xt", [P, F + 2], f32) as xt,
            nc.sbuf_tensor("et", [P, F], f32) as et,
            nc.sbuf_tensor("ps", [P, 1], f32) as ps,
            nc.sbuf_tensor("res", [1, 1], f32) as res,
            nc.psum_tensor("den", [1, 1], f32) as den,
        ):

            @block.sync
            def _(sync):
                sync.dma_start(xt[:, :], xin[:, :]).then_inc(dma_sem, 16)
                sync.wait_ge(dve_sem, 1)
                sync.dma_start(out[:, :], res[:, :]).then_inc(out_sem, 16)
                sync.wait_ge(out_sem, 16)

            @block.vector
            def _(vector):
                vector.wait_ge(pe_sem, 1)
                vector.reciprocal(res[:, :], den[:, :]).then_inc(dve_sem, 1)

            @block.scalar
            def _(scalar):
                scalar.wait_ge(dma_sem, 16)
                # et = exp(x_j - x_t); ps[p] = this partition's row sum
                scalar.activation(
                    et[:, :],
                    xt[:, 1 : F + 1],
                    mybir.ActivationFunctionType.Exp,
                    bias=xt[:, 0:1],
                    accum_out=ps[:, :],
                ).then_inc(act_sem, 1)

            @block.tensor
            def _(tensor):
                tensor.wait_ge(act_sem, 1)
                # den = ones^T @ ps = sum over partitions
                tensor.matmul(
                    den[:, :],
                    ps[:, :],
                    xt[:, F + 1 : F + 2],
                    start=True,
                    stop=True,
                ).then_inc(pe_sem, 1)

        nc.compile()
    finally:
        bass.Bass.all_engine_barrier = orig_barrier
    return nc


def _get_nc():
    if "nc" not in _CACHE:
        _CACHE["nc"] = _build()
    return _CACHE["nc"]


def _prep_inputs(x, label):
    if isinstance(x, np.ndarray) and isinstance(label, np.ndarray):
        l0 = int(label[0])
        row = np.asarray(x[0], dtype=np.float32).reshape(-1)
    else:
        # jax arrays: slice on CPU when uncommitted so an accidental
        # default-device dispatch doesn't ship the full 1GB to a NeuronCore;
        # committed device arrays slice in place and move only 128KB.
        try:
            import jax

            cpu = jax.local_devices(backend="cpu")[0]
            with jax.default_device(cpu):
                l0 = int(np.asarray(label[0]))
                row = np.asarray(x[0], dtype=np.float32).reshape(-1)
        except Exception:
            l0 = int(np.asarray(label[0]))
            row = np.asarray(x[0], dtype=np.float32).reshape(-1)
    buf = np.empty((P, F + 2), dtype=np.float32)
    buf[:, 1 : F + 1] = row.reshape(P, F)
    buf[:, 0] = -row[l0]
    buf[:, F + 1] = 1.0
    return {"xin": buf}


def _enable_jit_cache():
    # Persistent XLA cache: repeat kernel() calls skip the client-side
    # BIR->NEFF rebuild (~halves warm-call latency).  Best effort.
    if _CACHE.get("jit_cache"):
        return
    _CACHE["jit_cache"] = True
    try:
        import jax

        jax.config.update("jax_compilation_cache_dir", "/tmp/jax_neff_cache")
        jax.config.update("jax_persistent_cache_min_compile_time_secs", 0.0)
        jax.config.update("jax_persistent_cache_min_entry_size_bytes", 0)
    except Exception:
        pass


def _run(in_map, **kw):
    _enable_jit_cache()
    nc = _get_nc()
    return run_bass_kernel_spmd(nc, [in_map] * N_CORES, list(range(N_CORES)), **kw)


def kernel(x, label):
    in_map = _prep_inputs(x, label)
    try:
        res = _run(in_map)
    except Exception:
        res = _run(in_map)  # one retry: axon workers occasionally fault
    return np.float32(res.results[0]["out"][0, 0])


if __name__ == "__main__":
    rng = np.random.default_rng(0)
    x = rng.standard_normal((M, N), dtype=np.float32)
    label = rng.integers(0, N, size=(M,)).astype(np.int64)
    got = kernel(x, label)
    e = np.exp(x[0].astype(np.float64))
    want = e[label[0]] / e.sum()
    print("got", got, "want", want, "rel", abs(got - want) / abs(want))


# revision 13
# speedup vs baseline: 1.1830x; 1.1830x over previous
"""CrossEntropyLoss kernel for TRN2 (8 NeuronCores, SPMD).

The reference computes softmax over the full [8192, 32000] matrix but returns
only probs[0, label[0]] = exp(x[0, l0]) / sum_j exp(x[0, j]) -- a scalar that
depends only on row 0 of x and label[0].  The kernel therefore streams just
row 0 (32000 f32, 128KB) to the cores instead of the full 1GB, computes
exp(x_j - x_t) with the target shift folded into the activation bias, row-sums
via the activation accumulator, cross-partition-sums via a ones-matmul on the
PE, and takes the reciprocal on the DVE -- that value is the answer, emitted
by core 0.  Exact for any input; differences from the reference are float32
rounding only.

Latency structure (instruction-cost-model, ~4.8us/core):
 - No all-engine start/end barriers: they only order the const-pool memsets
   (unused here; the bias / ones columns ship inside the input DMA) and align
   engine halts.  All ordering is explicit semaphores.
 - The 4-byte result leaves via a SWDGE scatter descriptor prepared on the
   GpSimd ring during the input DMA's dead time and fired by trigger_dma
   right after the reciprocal -- skipping the post-compute HWDGE generation
   (625ns) and DGE start delay (650ns) of a plain dma_start.  Each of the
   128 src rows scatters to its own out slot (row 0 = result, rows 1..127 =
   zeros); the runtime hands the kernel a zeroed output buffer, so
   scatter-add == write.  gpsimd.drain() quiesces the ring before halt.

Self-contained: hardcodes shapes, imports only installed packages.
"""

import numpy as np

import concourse.bass as bass
import concourse.bacc as bacc
import concourse.mybir as mybir
from concourse.bass_utils import run_bass_kernel_spmd

M, N = 8192, 32000
P, F = 128, 250  # row 0 reshaped to [128 partitions, 250 free]
N_CORES = 8
ELEM = 64  # 64 f32 = 256B, the scatter stride granularity

_CACHE = {}


def _build():
    orig_barrier = bass.Bass.all_engine_barrier
    bass.Bass.all_engine_barrier = lambda self, *, sem_only=False: None
    try:
        nc = bacc.Bacc(
            "TRN2",
            target_bir_lowering=False,
            debug=False,
            num_devices=N_CORES,
        )
        f32 = mybir.dt.float32
        i16 = mybir.dt.int16
        # Input layout: col 0 = -x[0, label[0]] (per-partition activation
        # bias); cols 1..250 = row 0 of x; col 251 = 1.0 (PE reduce vector).
        xin = nc.dram_tensor("xin", [P, F + 2], f32, kind="ExternalInput")
        out = nc.dram_tensor("out", [256, ELEM], f32, kind="ExternalOutput")

        with (
            nc.Block() as block,
            nc.semaphore("dma_sem") as dma_sem,
            nc.semaphore("act_sem") as act_sem,
            nc.semaphore("pe_sem") as pe_sem,
            nc.semaphore("dve_sem") as dve_sem,
            nc.semaphore("prep_sem") as prep_sem,
            nc.semaphore("idx_sem") as idx_sem,
            nc.semaphore("mset_sem") as mset_sem,
            nc.semaphore("out_sem") as out_sem,
            nc.sbuf_tensor("xt", [P, F + 2], f32) as xt,
            nc.sbuf_tensor("et", [P, F], f32) as et,
            nc.sbuf_tensor("ps", [P, 1], f32) as ps,
            nc.sbuf_tensor("src", [P, ELEM], f32) as src,
            nc.sbuf_tensor("idxs", [128, 8], i16) as idxs,
            nc.psum_tensor("den", [1, 1], f32) as den,
        ):

            @block.sync
            def _(sync):
                sync.dma_start(xt[:, :], xin[:, :]).then_inc(dma_sem, 16)

            @block.vector
            def _(vector):
                vector.memset(src[:, :], 0.0).then_inc(mset_sem, 1)
                vector.wait_ge(mset_sem, 1)
                vector.wait_ge(pe_sem, 1)
                vector.reciprocal(src[0:1, 0:1], den[:, :]).then_inc(dve_sem, 1)

            @block.scalar
            def _(scalar):
                scalar.wait_ge(dma_sem, 16)
                # et = exp(x_j - x_t); ps[p] = this partition's row sum
                scalar.activation(
                    et[:, :],
                    xt[:, 1 : F + 1],
                    mybir.ActivationFunctionType.Exp,
                    bias=xt[:, 0:1],
                    accum_out=ps[:, :],
                ).then_inc(act_sem, 1)

            @block.tensor
            def _(tensor):
                tensor.wait_ge(act_sem, 1)
                # den = ones^T @ ps = sum over partitions
                tensor.matmul(
                    den[:, :],
                    ps[:, :],
                    xt[:, F + 1 : F + 2],
                    start=True,
                    stop=True,
                ).then_inc(pe_sem, 1)

            @block.gpsimd
            def _(gpsimd):
                # wrapped idx layout [p, s] = p + 16*s: logical idx i lives
                # at [i % 16, i // 16] -> each src row i scatters to out
                # slot i; row 0 carries the result, rows 1..127 zeros.
                gpsimd.iota(
                    idxs[:, :], [[16, 8]], base=0, channel_multiplier=1
                ).then_inc(idx_sem, 1)
                gpsimd.wait_ge(idx_sem, 1)
                gpsimd.dma_scatter_add(
                    out[:, :],
                    bass.AP(src, 0, [[src.ap().ap[0][0], P], [ELEM, 1], [1, ELEM]]),
                    idxs[:, :],
                    num_idxs=128,
                    num_idxs_reg=128,
                    elem_size=ELEM,
                    prepare_only=True,
                    sem=out_sem,
                ).then_inc(prep_sem, 1)
                gpsimd.wait_ge(prep_sem, 1)
                gpsimd.wait_ge(dve_sem, 1)
                gpsimd.trigger_dma(1)
                gpsimd.wait_ge(out_sem, 16)
                gpsimd.drain()  # SWDGE quiesce before halt

        nc.compile()
    finally:
        bass.Bass.all_engine_barrier = orig_barrier
    return nc


def _get_nc():
    if "nc" not in _CACHE:
        _CACHE["nc"] = _build()
    return _CACHE["nc"]


def _prep_inputs(x, label):
    if isinstance(x, np.ndarray) and isinstance(label, np.ndarray):
        l0 = int(label[0])
        row = np.asarray(x[0], dtype=np.float32).reshape(-1)
    else:
        # jax arrays: slice on CPU when uncommitted so an accidental
        # default-device dispatch doesn't ship the full 1GB to a NeuronCore;
        # committed device arrays slice in place and move only 128KB.
        try:
            import jax

            cpu = jax.local_devices(backend="cpu")[0]
            with jax.default_device(cpu):
                l0 = int(np.asarray(label[0]))
                row = np.asarray(x[0], dtype=np.float32).reshape(-1)
        except Exception:
            l0 = int(np.asarray(label[0]))
            row = np.asarray(x[0], dtype=np.float32).reshape(-1)
    buf = np.empty((P, F + 2), dtype=np.float32)
    buf[:, 1 : F + 1] = row.reshape(P, F)
    buf[:, 0] = -row[l0]
    buf[:, F + 1] = 1.0
    return {"xin": buf}


def _enable_jit_cache():
    # Persistent XLA cache: repeat kernel() calls skip the client-side
    # BIR->NEFF rebuild (~halves warm-call latency).  Best effort.
    if _CACHE.get("jit_cache"):
        return
    _CACHE["jit_cache"] = True
    try:
        import jax

        jax.config.update("jax_compilation_cache_dir", "/tmp/jax_neff_cache")
        jax.config.update("jax_persistent_cache_min_compile_time_secs", 0.0)
        jax.config.update("jax_persistent_cache_min_entry_size_bytes", 0)
    except Exception:
        pass


def _run(in_map, **kw):
    _enable_jit_cache()
    nc = _get_nc()
    return run_bass_kernel_spmd(nc, [in_map] * N_CORES, list(range(N_CORES)), **kw)


def kernel(x, label):
    in_map = _prep_inputs(x, label)
    try:
        res = _run(in_map)
    except Exception:
        res = _run(in_map)  # one retry: axon workers occasionally fault
    return np.float32(res.results[0]["out"][0, 0])


if __name__ == "__main__":
    rng = np.random.default_rng(0)
    x = rng.standard_normal((M, N), dtype=np.float32)
    label = rng.integers(0, N, size=(M,)).astype(np.int64)
    got = kernel(x, label)
    e = np.exp(x[0].astype(np.float64))
    want = e[label[0]] / e.sum()
    print("got", got, "want", want, "rel", abs(got - want) / abs(want))


# revision 14
# speedup vs baseline: 1.2234x; 1.0342x over previous
"""CrossEntropyLoss kernel for TRN2 (8 NeuronCores, SPMD).

The reference computes softmax over the full [8192, 32000] matrix but returns
only probs[0, label[0]] = exp(x[0, l0]) / sum_j exp(x[0, j]) -- a scalar that
depends only on row 0 of x and label[0].  The kernel therefore streams just
row 0 (32000 f32, 128KB) to the cores instead of the full 1GB, computes
exp(x_j - x_t) with the target shift folded into the activation bias, row-sums
via the activation accumulator, cross-partition-sums via a ones-matmul on the
PE, and takes the reciprocal on the DVE -- that value is the answer, emitted
by core 0.  Exact for any input; differences from the reference are float32
rounding only.

Latency structure (instruction-cost-model, ~4.8us/core):
 - No all-engine start/end barriers: they only order the const-pool memsets
   (unused here; the bias / ones columns ship inside the input DMA) and align
   engine halts.  All ordering is explicit semaphores.
 - The 4-byte result leaves via a SWDGE scatter descriptor prepared on the
   GpSimd ring during the input DMA's dead time and fired by trigger_dma
   right after the reciprocal -- skipping the post-compute HWDGE generation
   (625ns) and DGE start delay (650ns) of a plain dma_start.  Each of the
   128 src rows scatters to its own out slot (row 0 = result, rows 1..127 =
   zeros); the runtime hands the kernel a zeroed output buffer, so
   scatter-add == write.  gpsimd.drain() quiesces the ring before halt.

Self-contained: hardcodes shapes, imports only installed packages.
"""

import numpy as np

import concourse.bass as bass
import concourse.bacc as bacc
import concourse.mybir as mybir
from concourse.bass_utils import run_bass_kernel_spmd

M, N = 8192, 32000
P, F = 128, 250  # row 0 reshaped to [128 partitions, 250 free]
N_CORES = 8
ELEM = 64  # 64 f32 = 256B, the scatter stride granularity

_CACHE = {}


def _build():
    orig_barrier = bass.Bass.all_engine_barrier
    bass.Bass.all_engine_barrier = lambda self, *, sem_only=False: None
    try:
        nc = bacc.Bacc(
            "TRN2",
            target_bir_lowering=False,
            debug=False,
            num_devices=N_CORES,
        )
        f32 = mybir.dt.float32
        i16 = mybir.dt.int16
        # Input layout: col 0 = -x[0, label[0]] (per-partition activation
        # bias); cols 1..250 = row 0 of x; col 251 = 1.0 (PE reduce vector).
        xin = nc.dram_tensor("xin", [P, F + 2], f32, kind="ExternalInput")
        out = nc.dram_tensor("out", [128, ELEM], f32, kind="ExternalOutput")

        with (
            nc.Block() as block,
            nc.semaphore("dma_sem") as dma_sem,
            nc.semaphore("act_sem") as act_sem,
            nc.semaphore("pe_sem") as pe_sem,
            nc.semaphore("dve_sem") as dve_sem,
            nc.semaphore("prep_sem") as prep_sem,
            nc.semaphore("idx_sem") as idx_sem,
            nc.semaphore("mset_sem") as mset_sem,
            nc.semaphore("out_sem") as out_sem,
            nc.sbuf_tensor("xt", [P, F + 2], f32) as xt,
            nc.sbuf_tensor("et", [P, F], f32) as et,
            nc.sbuf_tensor("ps", [P, 1], f32) as ps,
            nc.sbuf_tensor("src", [P, ELEM], f32) as src,
            nc.sbuf_tensor("idxs", [128, 1], i16) as idxs,
            nc.psum_tensor("den", [1, 1], f32) as den,
        ):

            @block.sync
            def _(sync):
                sync.dma_start(xt[:, :], xin[:, :]).then_inc(dma_sem, 16)

            @block.vector
            def _(vector):
                vector.memset(src[:, :], 0.0).then_inc(mset_sem, 1)
                vector.wait_ge(mset_sem, 1)
                vector.wait_ge(pe_sem, 1)
                vector.reciprocal(src[0:1, 0:1], den[:, :]).then_inc(dve_sem, 1)

            @block.scalar
            def _(scalar):
                scalar.wait_ge(dma_sem, 16)
                # et = exp(x_j - x_t); ps[p] = this partition's row sum
                scalar.activation(
                    et[:, :],
                    xt[:, 1 : F + 1],
                    mybir.ActivationFunctionType.Exp,
                    bias=xt[:, 0:1],
                    accum_out=ps[:, :],
                ).then_inc(act_sem, 1)

            @block.tensor
            def _(tensor):
                tensor.wait_ge(act_sem, 1)
                # den = ones^T @ ps = sum over partitions
                tensor.matmul(
                    den[:, :],
                    ps[:, :],
                    xt[:, F + 1 : F + 2],
                    start=True,
                    stop=True,
                ).then_inc(pe_sem, 1)

            @block.gpsimd
            def _(gpsimd):
                # idx[p] = p: 16 logical idxs (partitions 0..15) -> only 16
                # descriptors; rows 0..15 scatter to slots 0..15 (row 0 =
                # result, rest zeros).  Partitions 16..127 are bounds-checked
                # only (p < 128 slots).
                gpsimd.iota(
                    idxs[:, :], [[0, 1]], base=0, channel_multiplier=1
                ).then_inc(idx_sem, 1)
                gpsimd.wait_ge(idx_sem, 1)
                gpsimd.dma_scatter_add(
                    out[:, :],
                    bass.AP(src, 0, [[src.ap().ap[0][0], P], [ELEM, 1], [1, ELEM]]),
                    idxs[:, :],
                    num_idxs=16,
                    num_idxs_reg=16,
                    elem_size=ELEM,
                    prepare_only=True,
                    sem=out_sem,
                ).then_inc(prep_sem, 1)
                gpsimd.wait_ge(prep_sem, 1)
                gpsimd.wait_ge(dve_sem, 1)
                gpsimd.trigger_dma(1)
                gpsimd.wait_ge(out_sem, 16)
                gpsimd.drain()  # SWDGE quiesce before halt

        nc.compile()
    finally:
        bass.Bass.all_engine_barrier = orig_barrier
    return nc


def _get_nc():
    if "nc" not in _CACHE:
        _CACHE["nc"] = _build()
    return _CACHE["nc"]


def _prep_inputs(x, label):
    if isinstance(x, np.ndarray) and isinstance(label, np.ndarray):
        l0 = int(label[0])
        row = np.asarray(x[0], dtype=np.float32).reshape(-1)
    else:
        # jax arrays: slice on CPU when uncommitted so an accidental
        # default-device dispatch doesn't ship the full 1GB to a NeuronCore;
        # committed device arrays slice in place and move only 128KB.
        try:
            import jax

            cpu = jax.local_devices(backend="cpu")[0]
            with jax.default_device(cpu):
                l0 = int(np.asarray(label[0]))
                row = np.asarray(x[0], dtype=np.float32).reshape(-1)
        except Exception:
            l0 = int(np.asarray(label[0]))
            row = np.asarray(x[0], dtype=np.float32).reshape(-1)
    buf = np.empty((P, F + 2), dtype=np.float32)
    buf[:, 1 : F + 1] = row.reshape(P, F)
    buf[:, 0] = -row[l0]
    buf[:, F + 1] = 1.0
    return {"xin": buf}


def _enable_jit_cache():
    # Persistent XLA cache: repeat kernel() calls skip the client-side
    # BIR->NEFF rebuild (~halves warm-call latency).  Best effort.
    if _CACHE.get("jit_cache"):
        return
    _CACHE["jit_cache"] = True
    try:
        import jax

        jax.config.update("jax_compilation_cache_dir", "/tmp/jax_neff_cache")
        jax.config.update("jax_persistent_cache_min_compile_time_secs", 0.0)
        jax.config.update("jax_persistent_cache_min_entry_size_bytes", 0)
    except Exception:
        pass


def _run(in_map, **kw):
    _enable_jit_cache()
    nc = _get_nc()
    return run_bass_kernel_spmd(nc, [in_map] * N_CORES, list(range(N_CORES)), **kw)


def kernel(x, label):
    in_map = _prep_inputs(x, label)
    try:
        res = _run(in_map)
    except Exception:
        res = _run(in_map)  # one retry: axon workers occasionally fault
    return np.float32(res.results[0]["out"][0, 0])


if __name__ == "__main__":
    rng = np.random.default_rng(0)
    x = rng.standard_normal((M, N), dtype=np.float32)
    label = rng.integers(0, N, size=(M,)).astype(np.int64)
    got = kernel(x, label)
    e = np.exp(x[0].astype(np.float64))
    want = e[label[0]] / e.sum()
    print("got", got, "want", want, "rel", abs(got - want) / abs(want))


# revision 15
# speedup vs baseline: 1.2437x; 1.0166x over previous
"""CrossEntropyLoss kernel for TRN2 (8 NeuronCores, SPMD).

The reference computes softmax over the full [8192, 32000] matrix but returns
only probs[0, label[0]] = exp(x[0, l0]) / sum_j exp(x[0, j]) -- a scalar that
depends only on row 0 of x and label[0].  The kernel therefore streams just
row 0 (32000 f32, 128KB) to the cores instead of the full 1GB, computes
exp(x_j - x_t) with the target shift folded into the activation bias, row-sums
via the activation accumulator, cross-partition-sums via a ones-matmul on the
PE, and takes the reciprocal on the DVE -- that value is the answer, emitted
by core 0.  Exact for any input; differences from the reference are float32
rounding only.

Latency structure (instruction-cost-model, ~4.8us/core):
 - No all-engine start/end barriers: they only order the const-pool memsets
   (unused here; the bias / ones columns ship inside the input DMA) and align
   engine halts.  All ordering is explicit semaphores.
 - The 4-byte result leaves via a SWDGE scatter descriptor prepared on the
   GpSimd ring during the input DMA's dead time and fired by trigger_dma
   right after the reciprocal -- skipping the post-compute HWDGE generation
   (625ns) and DGE start delay (650ns) of a plain dma_start.  Each of the
   128 src rows scatters to its own out slot (row 0 = result, rows 1..127 =
   zeros); the runtime hands the kernel a zeroed output buffer, so
   scatter-add == write.  gpsimd.drain() quiesces the ring before halt.

Self-contained: hardcodes shapes, imports only installed packages.
"""

import numpy as np

import concourse.bass as bass
import concourse.bacc as bacc
import concourse.mybir as mybir
from concourse.bass_utils import run_bass_kernel_spmd

M, N = 8192, 32000
P, F = 128, 250  # row 0 reshaped to [128 partitions, 250 free]
N_CORES = 8
ELEM = 64  # 64 f32 = 256B, the scatter stride granularity

_CACHE = {}


def _build():
    orig_barrier = bass.Bass.all_engine_barrier
    bass.Bass.all_engine_barrier = lambda self, *, sem_only=False: None
    try:
        nc = bacc.Bacc(
            "TRN2",
            target_bir_lowering=False,
            debug=False,
            num_devices=N_CORES,
        )
        f32 = mybir.dt.float32
        i16 = mybir.dt.int16
        # Input layout: col 0 = -x[0, label[0]] (per-partition activation
        # bias); cols 1..250 = row 0 of x; col 251 = 1.0 (PE reduce vector).
        xin = nc.dram_tensor("xin", [P, F + 2], f32, kind="ExternalInput")
        out = nc.dram_tensor("out", [128, ELEM], f32, kind="ExternalOutput")

        with (
            nc.Block() as block,
            nc.semaphore("dma_sem") as dma_sem,
            nc.semaphore("act_sem") as act_sem,
            nc.semaphore("pe_sem") as pe_sem,
            nc.semaphore("dve_sem") as dve_sem,
            nc.semaphore("prep_sem") as prep_sem,
            nc.semaphore("idx_sem") as idx_sem,
            nc.semaphore("mset_sem") as mset_sem,
            nc.semaphore("out_sem") as out_sem,
            nc.sbuf_tensor("xt", [P, F + 2], f32) as xt,
            nc.sbuf_tensor("et", [P, F], f32) as et,
            nc.sbuf_tensor("ps", [P, 1], f32) as ps,
            nc.sbuf_tensor("src", [P, ELEM], f32) as src,
            nc.sbuf_tensor("idxs", [128, 1], i16) as idxs,
            nc.sbuf_tensor("den_sb", [1, 1], f32) as den_sb,
        ):

            @block.sync
            def _(sync):
                sync.dma_start(xt[:, :], xin[:, :]).then_inc(dma_sem, 16)

            @block.vector
            def _(vector):
                vector.memset(src[:, :], 0.0).then_inc(mset_sem, 1)
                vector.wait_ge(mset_sem, 1)
                vector.wait_ge(pe_sem, 1)
                vector.reciprocal(src[0:1, 0:1], den_sb[0:1, 0:1]).then_inc(dve_sem, 1)

            @block.scalar
            def _(scalar):
                scalar.wait_ge(dma_sem, 16)
                # et = exp(x_j - x_t); ps[p] = this partition's row sum
                scalar.activation(
                    et[:, :],
                    xt[:, 1 : F + 1],
                    mybir.ActivationFunctionType.Exp,
                    bias=xt[:, 0:1],
                    accum_out=ps[:, :],
                ).then_inc(act_sem, 1)

            @block.gpsimd
            def _(gpsimd):
                # idx[p] = p: 16 logical idxs (partitions 0..15) -> only 16
                # descriptors; rows 0..15 scatter to slots 0..15 (row 0 =
                # result, rest zeros).  Partitions 16..127 are bounds-checked
                # only (p < 128 slots).
                gpsimd.iota(
                    idxs[:, :], [[0, 1]], base=0, channel_multiplier=1
                ).then_inc(idx_sem, 1)
                gpsimd.wait_ge(idx_sem, 1)
                gpsimd.dma_scatter_add(
                    out[:, :],
                    bass.AP(src, 0, [[src.ap().ap[0][0], P], [ELEM, 1], [1, ELEM]]),
                    idxs[:, :],
                    num_idxs=16,
                    num_idxs_reg=16,
                    elem_size=ELEM,
                    prepare_only=True,
                    sem=out_sem,
                ).then_inc(prep_sem, 1)
                gpsimd.wait_ge(act_sem, 1)
                # den = sum over partitions (C-axis reduce is gpsimd-only);
                # beats the PE matmul: no PSUM-write latency, no PE hop
                gpsimd.reduce_sum(
                    den_sb[0:1, 0:1], ps[:, :], axis=mybir.AxisListType.C
                ).then_inc(pe_sem, 1)
                gpsimd.wait_ge(prep_sem, 1)
                gpsimd.wait_ge(dve_sem, 1)
                gpsimd.trigger_dma(1)
                gpsimd.wait_ge(out_sem, 16)
                gpsimd.drain()  # SWDGE quiesce before halt

        nc.compile()
    finally:
        bass.Bass.all_engine_barrier = orig_barrier
    return nc


def _get_nc():
    if "nc" not in _CACHE:
        _CACHE["nc"] = _build()
    return _CACHE["nc"]


def _prep_inputs(x, label):
    if isinstance(x, np.ndarray) and isinstance(label, np.ndarray):
        l0 = int(label[0])
        row = np.asarray(x[0], dtype=np.float32).reshape(-1)
    else:
        # jax arrays: slice on CPU when uncommitted so an accidental
        # default-device dispatch doesn't ship the full 1GB to a NeuronCore;
        # committed device arrays slice in place and move only 128KB.
        try:
            import jax

            cpu = jax.local_devices(backend="cpu")[0]
            with jax.default_device(cpu):
                l0 = int(np.asarray(label[0]))
                row = np.asarray(x[0], dtype=np.float32).reshape(-1)
        except Exception:
            l0 = int(np.asarray(label[0]))
            row = np.asarray(x[0], dtype=np.float32).reshape(-1)
    buf = np.empty((P, F + 2), dtype=np.float32)
    buf[:, 1 : F + 1] = row.reshape(P, F)
    buf[:, 0] = -row[l0]
    buf[:, F + 1] = 1.0
    return {"xin": buf}


def _enable_jit_cache():
    # Persistent XLA cache: repeat kernel() calls skip the client-side
    # BIR->NEFF rebuild (~halves warm-call latency).  Best effort.
    if _CACHE.get("jit_cache"):
        return
    _CACHE["jit_cache"] = True
    try:
        import jax

        jax.config.update("jax_compilation_cache_dir", "/tmp/jax_neff_cache")
        jax.config.update("jax_persistent_cache_min_compile_time_secs", 0.0)
        jax.config.update("jax_persistent_cache_min_entry_size_bytes", 0)
    except Exception:
        pass


def _run(in_map, **kw):
    _enable_jit_cache()
    nc = _get_nc()
    return run_bass_kernel_spmd(nc, [in_map] * N_CORES, list(range(N_CORES)), **kw)


def kernel(x, label):
    in_map = _prep_inputs(x, label)
    try:
        res = _run(in_map)
    except Exception:
        res = _run(in_map)  # one retry: axon workers occasionally fault
    return np.float32(res.results[0]["out"][0, 0])


if __name__ == "__main__":
    rng = np.random.default_rng(0)
    x = rng.standard_normal((M, N), dtype=np.float32)
    label = rng.integers(0, N, size=(M,)).astype(np.int64)
    got = kernel(x, label)
    e = np.exp(x[0].astype(np.float64))
    want = e[label[0]] / e.sum()
    print("got", got, "want", want, "rel", abs(got - want) / abs(want))
